# revision 16
# baseline (speedup 1.0000x reference)
"""Trainium2 Bass kernel for nn_NeuronS3DiffUpsample2D.

Reference computation (per sample b):
    up   = nearest-2x-upsample(x[b])                       # [C, 320, 320]
    w    = Wb + 0.25 * einsum('or,rikl->oikl', lora_up, lora_down)
    w_b  = w * de_mod[b, None, :, None, None]              # modulate input chans
    dem  = rsqrt(sum_{i,k,l} w_b^2 + eps)                  # per output chan
    y[b] = conv2d(up, w_b * dem, SAME) + bias

Key algebraic transform: a 3x3 SAME conv on a 2x nearest-upsampled image
decomposes into 4 output phases (di, dj in {0,1}), each a 2x2 conv on the
ORIGINAL 160x160 input:
    y[2i+di, 2j+dj] = sum_{a,b in {0,1}} K[di,dj,a,b] @ x[i+a+di-1, j+b+dj-1]
where each K[di,dj,a,b] is a row-combo x col-combo sum of the 9 taps of w:
  row-combos (di,a): {w0, w1+w2, w0+w1, w2} over ki; same pattern over kj.
This is 4/9 of the naive FLOPs and never materializes the upsampled image.

Since the demod scale is per output channel and conv is linear in w, the conv
OUTPUT is scaled by dem[o] (per-partition scalar) at PSUM eviction, fused with
the bias add; weights are only modulated by de_mod on the input-channel axis.

Sharding: data-parallel over batch B=8 across 8 NeuronCores; each core builds
its own per-sample weights locally (replicated W/lora are tiny).

Performance notes (from perfetto traces of earlier revisions):
  * The conv loop is a zero-gap matmul stream; its cadence was set by f32r
    LDWEIGHTS (224 ns > the 200 ns N=480 matmul).  All matmul operands are
    bf16 now: LDWEIGHTS takes ~107 ns (with FWL) and hides fully, and the
    input DMA bytes halve.  Accumulation stays fp32 in PSUM; rel err ~2e-3
    against the fp32 reference.
  * x is padded to [C,162,162] with a zero border ON HOST so every band DMA
    is a single contiguous descriptor per partition (no SWDGE descriptor
    storms, no DVE border memsets) and arrives fast.
  * Of the 16 combined-tap matrices, 8 are direct views into the row-combo
    tiles (no copies); only the 8 column-sums are materialized by DVE.
  * The demod reduction uses 4 contiguous DVE adds instead of one strided
    tensor_reduce; its tiny PE matmul is scheduled before the conv stream so
    the PSUM pool for the conv loop can own all 8 banks.
"""

import sys
import numpy as np
from contextlib import ExitStack

try:
    import concourse.bass as bass
except ImportError:  # grading env without the axon PYTHONPATH
    sys.path.insert(0, "/opt/trn_rl_repo")
    import concourse.bass as bass
import concourse.tile as tile
from concourse import bacc, mybir
from concourse.bass_utils import run_bass_kernel_spmd

B, C, H, W = 8, 128, 160, 160
RANK = 32
SCALING = 0.25
EPS = 1e-8
HP, WP = H + 2, W + 2   # zero-padded image (1-px border baked in on host)
R_BLK = 3               # x-rows per matmul block -> N = 3*160 = 480 <= 512
C9 = 9 * C
NCORES = 8

# Input bands (padded-row ranges).  Block i0 needs padded rows [i0, i0+4];
# bands overlap by 4 rows so any block reads from a single tile.  The first
# band is small so the conv stream can start as soon as the weight stage is
# done; later bands are large to amortize DMA setup.
BANDS = [(0, 14), (12, 38), (36, 74), (72, 122), (120, 162)]

f32 = mybir.dt.float32
bf16 = mybir.dt.bfloat16


def _band_of(i0):
    if i0 <= 9:
        return 0
    if i0 <= 33:
        return 1
    if i0 <= 69:
        return 2
    if i0 <= 117:
        return 3
    return 4


def _conv_kernel(ctx, tc, y, x, wpk, lor):
    nc = tc.nc
    AF = mybir.ActivationFunctionType
    ALU = mybir.AluOpType

    const = ctx.enter_context(tc.tile_pool(name="const", bufs=1))

    demP = const.tile([128, 1], f32)         # rsqrt demod, per output chan
    evb = const.tile([128, 1], f32)          # bias[o], f32 for evictions
    dmf = const.tile([128, 1], f32)          # de_mod[i], f32 scalar operand
    wm3 = const.tile([128, C9], bf16)        # modulated 9-tap weights [i,(t o)]
    R01 = const.tile([128, 3 * C], bf16)     # row-combo ki1+ki2
    R10 = const.tile([128, 3 * C], bf16)     # row-combo ki0+ki1
    cmb = const.tile([128, 4, 2, C], bf16)   # col-sums per (di,a): [A=kj1+kj2, B=kj0+kj1]
    W9 = const.tile([128, C9 + 2], bf16)     # Wb^T [i,(t o)] + de_mod col + bias col

    # x bands: contiguous 1-descriptor-per-partition DMAs on the otherwise
    # idle GpSimd queue (separate from the weight DMAs on sync and the
    # output DMAs on sync).  band0 is issued immediately; bands 1-4 are
    # held behind a probe op that depends on the W9 weight DMA so their
    # bulk transfers don't steal SDMA engines from the weight stage.
    band_tiles = []
    for bi, (s, e) in enumerate(BANDS):
        bt = const.tile([128, e - s, WP], bf16, name=f"band{bi}")
        band_tiles.append((bt, s))

    dmv = W9[:, C9 : C9 + 1]                 # de_mod[i] per partition
    biasv = W9[:, C9 + 1 : C9 + 2]

    wtmp = ctx.enter_context(tc.tile_pool(name="wtmp", bufs=1))
    with tc.tile_pool(name="wpsum", bufs=1, space="PSUM") as wpsum:
        nc.sync.dma_start(W9[:], wpk[:])
        LOR = wtmp.tile([RANK, 10 * C], bf16)    # [lora_down^T | 0.25*lora_up^T]
        nc.sync.dma_start(LOR[:], lor[:])

        wprobe = wtmp.tile([1, 1], bf16)
        nc.gpsimd.tensor_copy(wprobe[:], W9[0:1, 0:1])
        for bi in range(len(BANDS)):
            bt, s = band_tiles[bi]
            nc.gpsimd.dma_start(bt[:], x[:, s : BANDS[bi][1], :])

        nc.vector.tensor_copy(evb[:], biasv)
        nc.vector.tensor_copy(dmf[:], dmv)
        dm2 = wtmp.tile([128, 1], bf16)          # de_mod[i]^2, demod matmul rhs
        nc.scalar.square(dm2[:], dmf[:])

        # deltaT_scaled[i, t, o] = 0.25 * sum_r down[r,i,t] * up[o,r];
        # wsum = Wb^T + deltaT (unmodulated)
        deltaP = wpsum.tile([128, C9], f32)
        wsum = wtmp.tile([128, C9], bf16)
        for t in range(9):
            nc.tensor.matmul(
                deltaP[:, t * C : (t + 1) * C],
                LOR[:, t * C : (t + 1) * C],
                LOR[:, 9 * C : 10 * C],
                start=True,
                stop=True,
            )
        # throwaway matmuls that keep the PE busy through the weight-stage
        # DVE/ACT chain: the HAM clock gate needs ~3.4us of sustained PE
        # activity to lift the 1.2GHz cold throttle, so the conv stream
        # starts at the full 2.4GHz instead of paying a cold-ramp.  They
        # depend only on LOR so they chain directly after the delta MMs.
        warmP = wpsum.tile([128, 480], f32)
        for t in range(7):
            nc.tensor.matmul(
                warmP[:], LOR[:, 0:C], LOR[:, 0:480], start=True, stop=True
            )
        nc.vector.tensor_add(wsum[:], W9[:, 0:C9], deltaP[:])

    # wpsum (deltaP, warmP) is closed here so the conv PSUM pool can reuse
    # its banks; everything below only touches SBUF or its own PSUM pools.

    # wm3 = wsum * de_mod[i]; then row-combos and the 8 column-sum taps
    # (the other 8 of the 16 combined taps are direct views into
    # wm3/R01/R10).  Emitted before the demod reduce tree so the conv
    # stream (gated on these weights) starts as early as possible.
    nc.vector.tensor_scalar_mul(wm3[:], wsum[:], dmf[:, 0:1])
    nc.vector.tensor_add(R01[:], wm3[:, 3 * C : 6 * C], wm3[:, 6 * C : C9])
    nc.vector.tensor_add(R10[:], wm3[:, 0 : 3 * C], wm3[:, 3 * C : 6 * C])

    rcs = {
        (0, 0): (wm3, 0),
        (0, 1): (R01, 0),
        (1, 0): (R10, 0),
        (1, 1): (wm3, 6 * C),
    }
    for i, (di, a) in enumerate([(0, 0), (0, 1), (1, 0), (1, 1)]):
        tl, base = rcs[(di, a)]
        nc.vector.tensor_add(
            cmb[:, i, 0, :],
            tl[:, base + C : base + 2 * C],
            tl[:, base + 2 * C : base + 3 * C],
        )
        nc.vector.tensor_add(
            cmb[:, i, 1, :],
            tl[:, base : base + C],
            tl[:, base + C : base + 2 * C],
        )

    # demod: S = sum_t wsum^2 via ACT square + a tree of contiguous DVE
    # adds; then sum_i dm^2*S via a tiny N=1 matmul (dm^2 as the rhs).
    sq3 = wtmp.tile([128, C9], bf16)
    nc.scalar.square(sq3[:], wsum[:])
    a4 = wtmp.tile([128, 4 * C], bf16)
    nc.vector.tensor_add(a4[:], sq3[:, 0 : 4 * C], sq3[:, 4 * C : 8 * C])
    a2 = wtmp.tile([128, 2 * C], bf16)
    nc.vector.tensor_add(a2[:], a4[:, 0 : 2 * C], a4[:, 2 * C : 4 * C])
    s2t = wtmp.tile([128, C], bf16)
    nc.vector.tensor_add(s2t[:], a2[:, 0:C], a2[:, C : 2 * C])
    s2 = wtmp.tile([128, C], bf16)
    nc.vector.tensor_add(s2[:], s2t[:], sq3[:, 8 * C : C9])

    def lhsT_ap(di, dj, a, b):
        tl, base = rcs[(di, a)]
        if dj == 0 and b == 0:
            return tl[:, base : base + C]
        if dj == 1 and b == 1:
            return tl[:, base + 2 * C : base + 3 * C]
        return cmb[:, di * 2 + a, 0 if dj == 0 else 1, :]

    # ---- main conv loop ----
    # 7 PSUM banks for the conv phases + 1 for the tiny demod matmul (whose
    # PE slot is AFTER block0's matmuls, so conv start is gated only on the
    # combined-tap weights, not the demod reduce chain).
    spsum = ctx.enter_context(tc.tile_pool(name="spsum", bufs=1, space="PSUM"))
    mpsum = ctx.enter_context(tc.tile_pool(name="mpsum", bufs=7, space="PSUM"))
    opool = ctx.enter_context(tc.tile_pool(name="obuf", bufs=3))

    def emit_mms(i0, R):
        bt, s = band_tiles[_band_of(i0)]
        ph = []
        for p in range(4):
            di, dj = p >> 1, p & 1
            pt = mpsum.tile([128, R * W], f32, tag="ph", name=f"ph{p}_{i0}")
            for q in range(4):
                a, b = q >> 1, q & 1
                r0 = i0 + a + di - s         # padded row within band tile
                rhs = bt[:, r0 : r0 + R, b + dj : b + dj + W]
                nc.tensor.matmul(
                    pt[:], lhsT_ap(di, dj, a, b), rhs,
                    start=(q == 0), stop=(q == 3),
                )
            ph.append(pt)
        return ph

    def emit_evict(i0, R, ph):
        # interleave phases into full output rows; scale by demod, add bias
        ob = opool.tile([128, R, 2, 2 * W], f32, tag="ob", name=f"ob_{i0}")
        obv = ob.rearrange("p r d (j two) -> p r d two j", two=2)
        for p in range(4):
            di, dj = p >> 1, p & 1
            dst = obv[:, :, di, dj, :]
            srcv = ph[p].rearrange("p (r j) -> p r j", r=R)
            if dj == 0:
                nc.vector.tensor_scalar(
                    dst, srcv, demP[:, 0:1], evb[:, 0:1],
                    op0=ALU.mult, op1=ALU.add,
                )
            else:
                nc.scalar.activation(
                    dst, srcv, AF.Identity, bias=evb[:, 0:1], scale=demP[:, 0:1]
                )
        nc.sync.dma_start(y[:, 2 * i0 : 2 * i0 + 2 * R, :], ob[:])

    # R=3 blocks for rows 0..155, then four R=1 blocks: the small final
    # output DMAs (328KB vs 983KB) drain inline with the last matmuls
    # instead of leaving a ~3us backlog after the PE goes idle.
    blocks = [(i0, R_BLK) for i0 in range(0, 156, R_BLK)]
    blocks += [(i0, 1) for i0 in range(156, H)]

    ph0 = emit_mms(*blocks[0])

    sP = spsum.tile([128, 1], f32)
    nc.tensor.matmul(sP[:], s2[:], dm2[:], start=True, stop=True)
    t1 = wtmp.tile([128, 1], f32)
    nc.vector.tensor_scalar_add(t1[:], sP[:], EPS)
    t2 = wtmp.tile([128, 1], f32)
    nc.scalar.sqrt(t2[:], t1[:])
    nc.vector.reciprocal(demP[:], t2[:])

    emit_evict(*blocks[0], ph0)
    for i0, R in blocks[1:]:
        emit_evict(i0, R, emit_mms(i0, R))


def _build():
    nc = bacc.Bacc(
        "TRN2",
        target_bir_lowering=False,
        debug=False,
        enable_asserts=False,
        num_devices=NCORES,
    )
    x = nc.dram_tensor("x", [C, HP, WP], bf16, kind="ExternalInput").ap()
    wpk = nc.dram_tensor("wpk", [C, C9 + 2], bf16, kind="ExternalInput").ap()
    lor = nc.dram_tensor("lor", [RANK, 10 * C], bf16, kind="ExternalInput").ap()
    y = nc.dram_tensor("y", [C, 2 * H, 2 * W], f32, kind="ExternalOutput").ap()

    with tile.TileContext(nc) as tc:
        with ExitStack() as ctx:
            _conv_kernel(ctx, tc, y, x, wpk, lor)
    nc.compile()
    return nc


_CACHE = {}


def _get_nc():
    if "nc" not in _CACHE:
        _CACHE["nc"] = _build()
    return _CACHE["nc"]


def _make_in_maps(x, de_mod, Wb, lora_up, lora_down, bias):
    BF = mybir.dt.np(bf16)
    x = np.asarray(x, dtype=np.float32)
    de_mod = np.asarray(de_mod, dtype=np.float32)
    Wb = np.asarray(Wb, dtype=np.float32)
    lora_up = np.asarray(lora_up, dtype=np.float32)
    lora_down = np.asarray(lora_down, dtype=np.float32)
    bias = np.asarray(bias, dtype=np.float32).reshape(C)

    # zero-pad x with a 1-px border; bf16
    xp = np.zeros((B, C, HP, WP), dtype=BF)
    xp[:, :, 1 : 1 + H, 1 : 1 + W] = x.astype(BF)

    # [O,I,3,3] -> [i, (t o)];  [R,C,3,3] -> [r, (t i)]
    wbT = np.ascontiguousarray(Wb.transpose(1, 2, 3, 0).reshape(C, C9))
    ld = lora_down.transpose(0, 2, 3, 1).reshape(RANK, C9)
    lu = SCALING * lora_up.T                    # [r, o], lora scale folded in
    lor = np.concatenate([ld, lu], axis=1).astype(BF)

    in_maps = []
    for b in range(NCORES):
        wpk = np.empty((C, C9 + 2), dtype=np.float32)
        wpk[:, 0:C9] = wbT
        wpk[:, C9] = de_mod[b]
        wpk[:, C9 + 1] = bias
        in_maps.append(
            {
                "x": np.ascontiguousarray(xp[b]),
                "wpk": wpk.astype(BF),
                "lor": lor,
            }
        )
    return in_maps


def run(inputs, trace=False, trace_kwargs=None):
    nc = _get_nc()
    in_maps = _make_in_maps(**inputs)
    res = run_bass_kernel_spmd(
        nc,
        in_maps,
        core_ids=list(range(NCORES)),
        trace=trace,
        **(trace_kwargs or {}),
    )
    y = np.stack([res.results[b]["y"] for b in range(NCORES)], axis=0)
    return y, res


def kernel(**inputs):
    y, _ = run(inputs)
    return y


# revision 20
# speedup vs baseline: 1.0055x; 1.0055x over previous
"""Trainium2 Bass kernel for nn_NeuronS3DiffUpsample2D.

Reference computation (per sample b):
    up   = nearest-2x-upsample(x[b])                       # [C, 320, 320]
    w    = Wb + 0.25 * einsum('or,rikl->oikl', lora_up, lora_down)
    w_b  = w * de_mod[b, None, :, None, None]              # modulate input chans
    dem  = rsqrt(sum_{i,k,l} w_b^2 + eps)                  # per output chan
    y[b] = conv2d(up, w_b * dem, SAME) + bias

Key algebraic transform: a 3x3 SAME conv on a 2x nearest-upsampled image
decomposes into 4 output phases (di, dj in {0,1}), each a 2x2 conv on the
ORIGINAL 160x160 input:
    y[2i+di, 2j+dj] = sum_{a,b in {0,1}} K[di,dj,a,b] @ x[i+a+di-1, j+b+dj-1]
where each K[di,dj,a,b] is a row-combo x col-combo sum of the 9 taps of w:
  row-combos (di,a): {w0, w1+w2, w0+w1, w2} over ki; same pattern over kj.
This is 4/9 of the naive FLOPs and never materializes the upsampled image.

Since the demod scale is per output channel and conv is linear in w, the conv
OUTPUT is scaled by dem[o] (per-partition scalar) at PSUM eviction, fused with
the bias add; weights are only modulated by de_mod on the input-channel axis.

Sharding: data-parallel over batch B=8 across 8 NeuronCores; each core builds
its own per-sample weights locally (replicated W/lora are tiny).

Performance notes (from perfetto traces of earlier revisions):
  * The conv loop is a zero-gap matmul stream; its cadence was set by f32r
    LDWEIGHTS (224 ns > the 200 ns N=480 matmul).  All matmul operands are
    bf16 now: LDWEIGHTS takes ~107 ns (with FWL) and hides fully, and the
    input DMA bytes halve.  Accumulation stays fp32 in PSUM; rel err ~2e-3
    against the fp32 reference.
  * x is padded to [C,162,162] with a zero border ON HOST so every band DMA
    is a single contiguous descriptor per partition (no SWDGE descriptor
    storms, no DVE border memsets) and arrives fast.
  * Of the 16 combined-tap matrices, 8 are direct views into the row-combo
    tiles (no copies); only the 8 column-sums are materialized by DVE.
  * The demod reduction uses 4 contiguous DVE adds instead of one strided
    tensor_reduce; its tiny PE matmul is scheduled before the conv stream so
    the PSUM pool for the conv loop can own all 8 banks.
"""

import sys
import numpy as np
from contextlib import ExitStack

try:
    import concourse.bass as bass
except ImportError:  # grading env without the axon PYTHONPATH
    sys.path.insert(0, "/opt/trn_rl_repo")
    import concourse.bass as bass
import concourse.tile as tile
from concourse import bacc, mybir
from concourse.bass_utils import run_bass_kernel_spmd

B, C, H, W = 8, 128, 160, 160
RANK = 32
SCALING = 0.25
EPS = 1e-8
HP, WP = H + 2, W + 2   # zero-padded image (1-px border baked in on host)
R_BLK = 3               # x-rows per matmul block -> N = 3*160 = 480 <= 512
C9 = 9 * C
NCORES = 8

# Input bands (padded-row ranges).  Block i0 needs padded rows [i0, i0+4];
# bands overlap by 4 rows so any block reads from a single tile.  The first
# band is small so the conv stream can start as soon as the weight stage is
# done; later bands are large to amortize DMA setup.
BANDS = [(0, 14), (12, 38), (36, 74), (72, 122), (120, 162)]

f32 = mybir.dt.float32
bf16 = mybir.dt.bfloat16


def _band_of(i0):
    if i0 <= 9:
        return 0
    if i0 <= 33:
        return 1
    if i0 <= 69:
        return 2
    if i0 <= 117:
        return 3
    return 4


def _conv_kernel(ctx, tc, y, x, wpk, lor):
    nc = tc.nc
    AF = mybir.ActivationFunctionType
    ALU = mybir.AluOpType

    const = ctx.enter_context(tc.tile_pool(name="const", bufs=1))

    demP = const.tile([128, 1], f32)         # rsqrt demod, per output chan
    evb = const.tile([128, 1], f32)          # bias[o], f32 for evictions
    dmf = const.tile([128, 1], f32)          # de_mod[i], f32 scalar operand
    wm3 = const.tile([128, C9], bf16)        # modulated 9-tap weights [i,(t o)]
    R01 = const.tile([128, 3 * C], bf16)     # row-combo ki1+ki2
    R10 = const.tile([128, 3 * C], bf16)     # row-combo ki0+ki1
    cmb = const.tile([128, 4, 2, C], bf16)   # col-sums per (di,a): [A=kj1+kj2, B=kj0+kj1]
    W9 = const.tile([128, C9 + 2], bf16)     # Wb^T [i,(t o)] + de_mod col + bias col

    # x bands: contiguous 1-descriptor-per-partition DMAs on the otherwise
    # idle GpSimd queue (separate from the weight DMAs on sync and the
    # output DMAs on sync).  band0 is issued immediately; bands 1-4 are
    # held behind a probe op that depends on the W9 weight DMA so their
    # bulk transfers don't steal SDMA engines from the weight stage.
    band_tiles = []
    for bi, (s, e) in enumerate(BANDS):
        bt = const.tile([128, e - s, WP], bf16, name=f"band{bi}")
        band_tiles.append((bt, s))

    dmv = W9[:, C9 : C9 + 1]                 # de_mod[i] per partition
    biasv = W9[:, C9 + 1 : C9 + 2]

    wtmp = ctx.enter_context(tc.tile_pool(name="wtmp", bufs=1))
    with tc.tile_pool(name="wpsum", bufs=1, space="PSUM") as wpsum:
        nc.sync.dma_start(W9[:], wpk[:])
        LOR = wtmp.tile([RANK, 10 * C], bf16)    # [lora_down^T | 0.25*lora_up^T]
        # scalar (ACT) HWDGE ring: runs in parallel with W9 on the sync ring
        nc.scalar.dma_start(LOR[:], lor[:])

        wprobe = wtmp.tile([1, 1], bf16)
        nc.gpsimd.tensor_copy(wprobe[:], W9[0:1, 0:1])
        for bi in range(len(BANDS)):
            bt, s = band_tiles[bi]
            nc.gpsimd.dma_start(bt[:], x[:, s : BANDS[bi][1], :])
        wsum = wtmp.tile([128, C9], bf16)

        nc.vector.tensor_copy(evb[:], biasv)
        nc.vector.tensor_copy(dmf[:], dmv)
        dm2 = wtmp.tile([128, 1], bf16)          # de_mod[i]^2, demod matmul rhs
        nc.scalar.square(dm2[:], dmf[:])

        # deltaT_scaled[i, t, o] = 0.25 * sum_r down[r,i,t] * up[o,r];
        # wsum = Wb^T + deltaT (unmodulated)
        deltaP = wpsum.tile([128, C9], f32)
        for t in range(9):
            nc.tensor.matmul(
                deltaP[:, t * C : (t + 1) * C],
                LOR[:, t * C : (t + 1) * C],
                LOR[:, 9 * C : 10 * C],
                start=True,
                stop=True,
            )
        # throwaway matmuls that keep the PE busy through the weight-stage
        # DVE/ACT chain: the HAM clock gate needs ~3.4us of sustained PE
        # activity to lift the 1.2GHz cold throttle, so the conv stream
        # starts at the full 2.4GHz instead of paying a cold-ramp.  They
        # depend only on LOR so they chain directly after the delta MMs.
        warmP = wpsum.tile([128, 480], f32)
        for t in range(7):
            nc.tensor.matmul(
                warmP[:], LOR[:, 0:C], LOR[:, 0:480], start=True, stop=True
            )
        nc.vector.tensor_add(wsum[:], W9[:, 0:C9], deltaP[:])
        nc.vector.tensor_scalar_mul(wm3[:], wsum[:], dmf[:, 0:1])

        # Row-combos on the (otherwise idle) GpSimd engine; the 8
        # column-sum taps on DVE, wm3-sourced ones first so the conv
        # stream can start while the R01/R10-sourced ones finish.  (The
        # other 8 of the 16 combined taps are direct views into
        # wm3/R01/R10.)
        nc.gpsimd.tensor_add(R01[:], wm3[:, 3 * C : 6 * C], wm3[:, 6 * C : C9])
        nc.gpsimd.tensor_add(R10[:], wm3[:, 0 : 3 * C], wm3[:, 3 * C : 6 * C])

        rcs = {
            (0, 0): (wm3, 0),
            (0, 1): (R01, 0),
            (1, 0): (R10, 0),
            (1, 1): (wm3, 6 * C),
        }
        for i, (di, a) in [(0, (0, 0)), (3, (1, 1)), (1, (0, 1)), (2, (1, 0))]:
            tl, base = rcs[(di, a)]
            nc.vector.tensor_add(
                cmb[:, i, 0, :],
                tl[:, base + C : base + 2 * C],
                tl[:, base + 2 * C : base + 3 * C],
            )
            nc.vector.tensor_add(
                cmb[:, i, 1, :],
                tl[:, base : base + C],
                tl[:, base + C : base + 2 * C],
            )

        # demod: S = sum_t wsum^2 (ACT square + GpSimd add tree) then the
        # tiny N=1 matmul against dm^2.  This all runs INSIDE the weight
        # stage: the first eviction -- which opens the HBM output stream,
        # the closing critical path -- needs demP.
        sq3 = wtmp.tile([128, C9], bf16)
        nc.scalar.square(sq3[:], wsum[:])
        a4 = wtmp.tile([128, 4 * C], bf16)
        nc.gpsimd.tensor_add(a4[:], sq3[:, 0 : 4 * C], sq3[:, 4 * C : 8 * C])
        a2 = wtmp.tile([128, 2 * C], bf16)
        nc.gpsimd.tensor_add(a2[:], a4[:, 0 : 2 * C], a4[:, 2 * C : 4 * C])
        s2t = wtmp.tile([128, C], bf16)
        nc.gpsimd.tensor_add(s2t[:], a2[:, 0:C], a2[:, C : 2 * C])
        s2 = wtmp.tile([128, C], bf16)
        nc.gpsimd.tensor_add(s2[:], s2t[:], sq3[:, 8 * C : C9])

        sP = wpsum.tile([128, 1], f32)
        nc.tensor.matmul(sP[:], s2[:], dm2[:], start=True, stop=True)
        t1 = wtmp.tile([128, 1], f32)
        nc.vector.tensor_scalar_add(t1[:], sP[:], EPS)
        t2 = wtmp.tile([128, 1], f32)
        nc.scalar.sqrt(t2[:], t1[:])
        nc.vector.reciprocal(demP[:], t2[:])

    # wpsum (deltaP, warmP, sP) is closed here so the conv PSUM pool can
    # reuse its banks.

    def lhsT_ap(di, dj, a, b):
        tl, base = rcs[(di, a)]
        if dj == 0 and b == 0:
            return tl[:, base : base + C]
        if dj == 1 and b == 1:
            return tl[:, base + 2 * C : base + 3 * C]
        return cmb[:, di * 2 + a, 0 if dj == 0 else 1, :]

    # ---- main conv loop ----
    mpsum = ctx.enter_context(tc.tile_pool(name="mpsum", bufs=8, space="PSUM"))
    opool = ctx.enter_context(tc.tile_pool(name="obuf", bufs=3))

    def emit_mms(i0, R):
        bt, s = band_tiles[_band_of(i0)]
        ph = []
        for p in range(4):
            di, dj = p >> 1, p & 1
            pt = mpsum.tile([128, R * W], f32, tag="ph", name=f"ph{p}_{i0}")
            for q in range(4):
                a, b = q >> 1, q & 1
                r0 = i0 + a + di - s         # padded row within band tile
                rhs = bt[:, r0 : r0 + R, b + dj : b + dj + W]
                nc.tensor.matmul(
                    pt[:], lhsT_ap(di, dj, a, b), rhs,
                    start=(q == 0), stop=(q == 3),
                )
            ph.append(pt)
        return ph

    def emit_evict(i0, R, ph):
        # interleave phases into full output rows; scale by demod, add bias
        ob = opool.tile([128, R, 2, 2 * W], f32, tag="ob", name=f"ob_{i0}")
        obv = ob.rearrange("p r d (j two) -> p r d two j", two=2)
        for p in range(4):
            di, dj = p >> 1, p & 1
            dst = obv[:, :, di, dj, :]
            srcv = ph[p].rearrange("p (r j) -> p r j", r=R)
            if dj == 0:
                nc.vector.tensor_scalar(
                    dst, srcv, demP[:, 0:1], evb[:, 0:1],
                    op0=ALU.mult, op1=ALU.add,
                )
            else:
                nc.scalar.activation(
                    dst, srcv, AF.Identity, bias=evb[:, 0:1], scale=demP[:, 0:1]
                )
        # alternate the two HWDGE rings so one ring's end-of-DMA completion
        # receipt (~1-2us) overlaps the other ring's data movement
        eng = nc.sync if (i0 // R_BLK) % 2 == 0 else nc.scalar
        eng.dma_start(y[:, 2 * i0 : 2 * i0 + 2 * R, :], ob[:])

    # Three R=1 blocks first (their small outputs start the HBM write
    # stream early, right as demP lands), R=3 for the body, and four R=1
    # blocks last (the small final DMAs drain inline with the last
    # matmuls instead of leaving a ~3us backlog after the PE goes idle).
    blocks = [(i0, 1) for i0 in range(0, 3)]
    blocks += [(i0, R_BLK) for i0 in range(3, 156, R_BLK)]
    blocks += [(i0, 1) for i0 in range(156, H)]

    for i0, R in blocks:
        emit_evict(i0, R, emit_mms(i0, R))


def _build():
    nc = bacc.Bacc(
        "TRN2",
        target_bir_lowering=False,
        debug=False,
        enable_asserts=False,
        num_devices=NCORES,
    )
    x = nc.dram_tensor("x", [C, HP, WP], bf16, kind="ExternalInput").ap()
    wpk = nc.dram_tensor("wpk", [C, C9 + 2], bf16, kind="ExternalInput").ap()
    lor = nc.dram_tensor("lor", [RANK, 10 * C], bf16, kind="ExternalInput").ap()
    y = nc.dram_tensor("y", [C, 2 * H, 2 * W], f32, kind="ExternalOutput").ap()

    with tile.TileContext(nc) as tc:
        with ExitStack() as ctx:
            _conv_kernel(ctx, tc, y, x, wpk, lor)
    nc.compile()
    return nc


_CACHE = {}


def _get_nc():
    if "nc" not in _CACHE:
        _CACHE["nc"] = _build()
    return _CACHE["nc"]


def _make_in_maps(x, de_mod, Wb, lora_up, lora_down, bias):
    BF = mybir.dt.np(bf16)
    x = np.asarray(x, dtype=np.float32)
    de_mod = np.asarray(de_mod, dtype=np.float32)
    Wb = np.asarray(Wb, dtype=np.float32)
    lora_up = np.asarray(lora_up, dtype=np.float32)
    lora_down = np.asarray(lora_down, dtype=np.float32)
    bias = np.asarray(bias, dtype=np.float32).reshape(C)

    # zero-pad x with a 1-px border; bf16
    xp = np.zeros((B, C, HP, WP), dtype=BF)
    xp[:, :, 1 : 1 + H, 1 : 1 + W] = x.astype(BF)

    # [O,I,3,3] -> [i, (t o)];  [R,C,3,3] -> [r, (t i)]
    wbT = np.ascontiguousarray(Wb.transpose(1, 2, 3, 0).reshape(C, C9))
    ld = lora_down.transpose(0, 2, 3, 1).reshape(RANK, C9)
    lu = SCALING * lora_up.T                    # [r, o], lora scale folded in
    lor = np.concatenate([ld, lu], axis=1).astype(BF)

    in_maps = []
    for b in range(NCORES):
        wpk = np.empty((C, C9 + 2), dtype=np.float32)
        wpk[:, 0:C9] = wbT
        wpk[:, C9] = de_mod[b]
        wpk[:, C9 + 1] = bias
        in_maps.append(
            {
                "x": np.ascontiguousarray(xp[b]),
                "wpk": wpk.astype(BF),
                "lor": lor,
            }
        )
    return in_maps


def run(inputs, trace=False, trace_kwargs=None):
    nc = _get_nc()
    in_maps = _make_in_maps(**inputs)
    res = run_bass_kernel_spmd(
        nc,
        in_maps,
        core_ids=list(range(NCORES)),
        trace=trace,
        **(trace_kwargs or {}),
    )
    y = np.stack([res.results[b]["y"] for b in range(NCORES)], axis=0)
    return y, res


def kernel(**inputs):
    y, _ = run(inputs)
    return y
